# revision 12
# baseline (speedup 1.0000x reference)
"""Trainium2 Bass kernel for DeformAxialDW.

Reference computes out = x + convH(x) + convW(x): depthwise 7-tap 1D convs
along H and W with fractional dilation r (bilinear sampling), which expand
into per-channel banded (Toeplitz) convs with 2S+1 integer taps,
S = floor(3*r)+1.

Layout/precision plan (per core = one batch item, 8 cores):
  - x is packed on the HOST to bf16 [2, 112+S, C, W]: two h-blocks with 2S
    rows of overlap (rows [0,112+S) and [112-S,224)).  The overlap lets each
    output block's H-conv be a single [112+S -> 112] banded matmul with NO
    edge/corner matmuls.
  - The identity (+x) is folded into the Toeplitz masters (+0.5 on the
    center tap of both the H and W masters), so out = Hconv' + Wconv'
    accumulates entirely in PSUM; no separate add pass.
  - One H master MH [112+2S, C, 112] is shared by both blocks via partition
    slices; one W master MW [112, C, 112+2S] is shared by both w-chunks via
    free-dim slices.
  - W-conv needs x transposed: 4 PE transposes per channel (bf16, via
    permutation matmul) -> PSUM -> one DVE copy to SBUF; the transposed
    chunks are the matmul *stationary* (stationary load is cheap), with the
    W master as the moving operand.
  - PSUM po tiles hold 2 channels padded to 256 f32 each (1 bank, no
    matmul bank crossing); f32->bf16 output copies run mostly on GpSimd
    (best cost/elem), every 4th on DVE.
  - Output bf16 [2, 112, C, W], unpacked + upcast on the host.
"""

import sys

import numpy as np

sys.path.insert(0, "/opt/trn_rl_repo")

import ml_dtypes

BF16 = ml_dtypes.bfloat16

C, H, W = 128, 224, 224
B = 8
HO = 112  # output rows per h-block

_CACHE = {}


def _tap_coeffs(w_taps: np.ndarray, r_val: float, S: int) -> np.ndarray:
    """Expand 7 fractional-dilation taps into 2S+1 integer-shift coeffs."""
    Cn, K = w_taps.shape
    P = K // 2
    alpha = np.zeros((Cn, 2 * S + 1), dtype=np.float64)
    for i in range(K):
        k_pos = i - P
        delta = np.float32(k_pos) * np.float32(r_val)
        d0 = int(np.floor(delta))
        frac = float(np.float32(delta) - np.float32(d0))
        alpha[:, d0 + S] += (1.0 - frac) * w_taps[:, i].astype(np.float64)
        alpha[:, d0 + 1 + S] += frac * w_taps[:, i].astype(np.float64)
    return alpha


def _banded(alpha: np.ndarray, rows: int, cols: int, diag_off: int, S: int):
    """M[i, c, jj] = alpha[c, (i - jj + diag_off) + S] where |i-jj+diag_off|<=S."""
    Cn = alpha.shape[0]
    out = np.zeros((rows, Cn, cols), dtype=np.float64)
    i = np.arange(rows)[:, None]
    jj = np.arange(cols)[None, :]
    d = i - jj + diag_off
    mask = np.abs(d) <= S
    ii, jjj = np.nonzero(mask)
    out[ii, :, jjj] = alpha[:, d[ii, jjj] + S].T
    return out


def _build_nc(S: int):
    import concourse.mybir as mybir
    from concourse import bacc
    from concourse.tile import TileContext

    f32 = mybir.dt.float32
    bf16 = mybir.dt.bfloat16
    fp8 = mybir.dt.float8e4

    HT = HO + 2 * S    # x tile rows per block incl. S zero-pad rows (124)
    MR = HO + 2 * S    # H master rows / W master cols (112+2S)
    WS = HO + S        # W-conv moving width per chunk (112+S)

    nc = bacc.Bacc("TRN2", target_bir_lowering=False, debug=False)
    x_p = nc.declare_dram_parameter("x", [2, HT, C, W], bf16, isOutput=False)
    mh_p = nc.declare_dram_parameter("mh", [MR, C, HO], bf16, isOutput=False)
    mw_p = nc.declare_dram_parameter("mw", [HO, C, MR], fp8, isOutput=False)
    id_p = nc.declare_dram_parameter("ident", [HT, HT], bf16, isOutput=False)
    out_p = nc.declare_dram_parameter("out", [2, HO, C, W], bf16, isOutput=True)

    G = 16  # channels per DMA / store group
    with TileContext(nc) as tc:
        with tc.tile_pool(name="const", bufs=1) as constp, \
             tc.tile_pool(name="xg", bufs=3) as xgp, \
             tc.tile_pool(name="xt", bufs=3) as xtp, \
             tc.tile_pool(name="og", bufs=3) as ogp, \
             tc.tile_pool(name="pp", bufs=2, space="PSUM") as ppp, \
             tc.tile_pool(name="po", bufs=3, space="PSUM") as pop:
            ident = constp.tile([HT, HT], bf16)
            nc.scalar.dma_start(out=ident[:, :], in_=id_p[:, :])
            mh = constp.tile([MR, C, HO], bf16, tag="mh")
            mw = constp.tile([HO, C, MR], fp8, tag="mw")
            ncopy = 0
            # small leading groups so the first matmuls start ~2us in
            # instead of waiting for a full 16-channel load
            sizes = [4, 4, 8] + [G] * ((C - 16) // G)
            c0 = 0
            for g, gs in enumerate(sizes):
                # chunked master loads so the first channels start early;
                # they ride the ACT HWDGE ring, x loads ride the SP ring
                nc.scalar.dma_start(out=mh[:, c0:c0 + gs, :], in_=mh_p[:, c0:c0 + gs, :])
                nc.scalar.dma_start(out=mw[:, c0:c0 + gs, :], in_=mw_p[:, c0:c0 + gs, :])
                xg = []
                for t in (0, 1):
                    xg_t = xgp.tile([HT, G, W], bf16, tag=f"xg{t}")
                    nc.sync.dma_start(
                        out=xg_t[:, 0:gs, :], in_=x_p[t, :, c0:c0 + gs, :]
                    )
                    xg.append(xg_t)
                og0 = ogp.tile([HO, G, W], bf16, tag="og0")
                og1 = ogp.tile([HO, G, W], bf16, tag="og1")
                og = [og0, og1]
                po = [None, None]

                def emit_transposes(cl):
                    # transpose both w-chunks of both blocks: pp[:, 2t+q, :]
                    pp = ppp.tile([HO, 4, HT], bf16, name=f"pp_{g}_{cl}", tag="pp")
                    for t in (0, 1):
                        for q in (0, 1):
                            nc.tensor.matmul(
                                out=pp[:, 2 * t + q, :],
                                lhsT=xg[t][0:HT, cl, q * HO:(q + 1) * HO],
                                rhs=ident[:, :],
                                is_transpose=True,
                                skip_group_check=True,
                            )
                    xt = xtp.tile([HO, 4, HO], bf16, name=f"xt_{g}_{cl}", tag="xt")
                    nc.vector.tensor_copy(out=xt[:, :, :], in_=pp[:, :, S:S + HO])
                    return xt

                # software pipeline: transposes run one channel ahead so the
                # PE never waits on the DVE PSUM->SBUF copy of x^T
                xt_next = emit_transposes(0)
                for cl in range(gs):
                    c = c0 + cl
                    xt = xt_next
                    if cl + 1 < gs:
                        xt_next = emit_transposes(cl + 1)
                    if cl % 2 == 0:
                        po_t0 = pop.tile([HO, 2, 256], f32, tag="po0")
                        po_t1 = pop.tile([HO, 2, 256], f32, tag="po1")
                        po = [po_t0, po_t1]
                    sl = cl % 2
                    for t in (0, 1):
                        # H-conv (+identity): banded [HT->HO] stationary,
                        # x block moving (zero pad rows contribute nothing)
                        nc.tensor.matmul(
                            out=po[t][:, sl, 0:W],
                            lhsT=mh[0:HT, c, :],
                            rhs=xg[t][0:HT, cl, :],
                            start=True, stop=False,
                        )
                    for t in (0, 1):
                        # W-conv: transposed-x stationary, W master moving;
                        # two w_in chunks
                        nc.tensor.matmul(
                            out=po[t][:, sl, 0:WS],
                            lhsT=xt[0:HO, 2 * t, :],
                            rhs=mw[0:HO, c, S:S + WS],
                            start=False, stop=False,
                        )
                        nc.tensor.matmul(
                            out=po[t][:, sl, HO - S:W],
                            lhsT=xt[0:HO, 2 * t + 1, :],
                            rhs=mw[0:HO, c, 0:WS],
                            start=False, stop=True,
                        )
                    if cl % 2 == 1:
                        for t in (0, 1):
                            src = po[t][:, :, 0:W]
                            dst = og[t][:, cl - 1:cl + 1, :]
                            # GPSIMD cannot read PSUM; balance DVE vs ACT
                            if ncopy % 7 == 6:
                                nc.vector.tensor_copy(out=dst, in_=src)
                            else:
                                nc.scalar.copy(out=dst, in_=src)
                            ncopy += 1
                for t in (0, 1):
                    # stores go through SWDGE on the otherwise-idle gpsimd
                    # queue so they block neither loads (SP) nor copies (ACT);
                    # sub-group chunks shorten the end-of-kernel store tail
                    last = g == len(sizes) - 1
                    cm = 4 if (gs <= 8 or last) else gs // 2
                    for hf in range(gs // cm):
                        eng = nc.scalar if last else nc.gpsimd
                        eng.dma_start(
                            out=out_p[t, :, c0 + hf * cm:c0 + (hf + 1) * cm, :],
                            in_=og[t][:, hf * cm:(hf + 1) * cm, :],
                        )
                c0 += gs
    nc.compile()
    return nc


def _prepare_consts(weight_h, weight_w, r):
    r_val = float(max(np.float32(r), np.float32(1.0)))
    S = int(np.floor(3.0 * r_val)) + 1
    assert S <= 16, f"dilation r={r_val} too large for this kernel (S={S})"
    wh = np.asarray(weight_h)[:, 0, :, 0].astype(np.float64)
    ww = np.asarray(weight_w)[:, 0, 0, :].astype(np.float64)
    ah = _tap_coeffs(wh, r_val, S)
    aw = _tap_coeffs(ww, r_val, S)
    # identity rides the H master only (bf16 keeps it at ~2^-9 precision;
    # the W master is fp8 where a folded identity would cost ~3%)
    ah[:, S] += 1.0
    mh = _banded(ah, HO + 2 * S, HO, -S, S).astype(BF16)
    mw = _banded(aw, HO, HO + 2 * S, S, S).astype(ml_dtypes.float8_e4m3fn)
    ident = np.eye(HO + 2 * S, dtype=BF16)
    return S, mh, mw, ident


def kernel(x, weight_h, weight_w, r):
    from concourse.bass_utils import run_bass_kernel_spmd

    x = np.asarray(x, dtype=np.float32)
    assert x.shape == (B, C, H, W), x.shape
    S, mh, mw, ident = _prepare_consts(weight_h, weight_w, r)
    HT = HO + 2 * S

    if S not in _CACHE:
        _CACHE[S] = _build_nc(S)
    nc = _CACHE[S]

    xb = x.astype(BF16)
    in_maps = []
    for b in range(B):
        pk = np.zeros((2, HT, C, W), dtype=BF16)
        pk[0, S:HT] = xb[b, :, 0:HO + S].transpose(1, 0, 2)
        pk[1, 0:HO + S] = xb[b, :, HO - S:H].transpose(1, 0, 2)
        in_maps.append({"x": pk, "mh": mh, "mw": mw, "ident": ident})

    res = run_bass_kernel_spmd(nc, in_maps, core_ids=list(range(B)))
    out = np.empty((B, C, H, W), dtype=np.float32)
    for b in range(B):
        o = np.asarray(res.results[b]["out"])  # (2, HO, C, W) bf16
        out[b, :, 0:HO] = o[0].transpose(1, 0, 2)
        out[b, :, HO:H] = o[1].transpose(1, 0, 2)
    return out


# revision 13
# speedup vs baseline: 1.0832x; 1.0832x over previous
"""Trainium2 Bass kernel for DeformAxialDW.

Reference computes out = x + convH(x) + convW(x): depthwise 7-tap 1D convs
along H and W with fractional dilation r (bilinear sampling), which expand
into per-channel banded (Toeplitz) convs with 2S+1 integer taps,
S = floor(3*r)+1.

Layout/precision plan (per core = one batch item, 8 cores):
  - x is packed on the HOST to bf16 [2, 112+S, C, W]: two h-blocks with 2S
    rows of overlap (rows [0,112+S) and [112-S,224)).  The overlap lets each
    output block's H-conv be a single [112+S -> 112] banded matmul with NO
    edge/corner matmuls.
  - The identity (+x) is folded into the Toeplitz masters (+0.5 on the
    center tap of both the H and W masters), so out = Hconv' + Wconv'
    accumulates entirely in PSUM; no separate add pass.
  - One H master MH [112+2S, C, 112] is shared by both blocks via partition
    slices; one W master MW [112, C, 112+2S] is shared by both w-chunks via
    free-dim slices.
  - W-conv needs x transposed: 4 PE transposes per channel (bf16, via
    permutation matmul) -> PSUM -> one DVE copy to SBUF; the transposed
    chunks are the matmul *stationary* (stationary load is cheap), with the
    W master as the moving operand.
  - PSUM po tiles hold 2 channels padded to 256 f32 each (1 bank, no
    matmul bank crossing); f32->bf16 output copies run mostly on GpSimd
    (best cost/elem), every 4th on DVE.
  - Output bf16 [2, 112, C, W], unpacked + upcast on the host.
"""

import sys

import numpy as np

sys.path.insert(0, "/opt/trn_rl_repo")

import ml_dtypes

BF16 = ml_dtypes.bfloat16

C, H, W = 128, 224, 224
B = 8
HO = 112  # output rows per h-block

_CACHE = {}


def _tap_coeffs(w_taps: np.ndarray, r_val: float, S: int) -> np.ndarray:
    """Expand 7 fractional-dilation taps into 2S+1 integer-shift coeffs."""
    Cn, K = w_taps.shape
    P = K // 2
    alpha = np.zeros((Cn, 2 * S + 1), dtype=np.float64)
    for i in range(K):
        k_pos = i - P
        delta = np.float32(k_pos) * np.float32(r_val)
        d0 = int(np.floor(delta))
        frac = float(np.float32(delta) - np.float32(d0))
        alpha[:, d0 + S] += (1.0 - frac) * w_taps[:, i].astype(np.float64)
        alpha[:, d0 + 1 + S] += frac * w_taps[:, i].astype(np.float64)
    return alpha


def _banded(alpha: np.ndarray, rows: int, cols: int, diag_off: int, S: int):
    """M[i, c, jj] = alpha[c, (i - jj + diag_off) + S] where |i-jj+diag_off|<=S."""
    Cn = alpha.shape[0]
    out = np.zeros((rows, Cn, cols), dtype=np.float64)
    i = np.arange(rows)[:, None]
    jj = np.arange(cols)[None, :]
    d = i - jj + diag_off
    mask = np.abs(d) <= S
    ii, jjj = np.nonzero(mask)
    out[ii, :, jjj] = alpha[:, d[ii, jjj] + S].T
    return out


def _build_nc(S: int):
    import concourse.mybir as mybir
    from concourse import bacc
    from concourse.tile import TileContext

    f32 = mybir.dt.float32
    bf16 = mybir.dt.bfloat16
    fp8 = mybir.dt.float8e4

    HT = HO + 2 * S    # x tile rows per block incl. S zero-pad rows (124)
    MR = HO + 2 * S    # H master rows / W master cols (112+2S)
    WS = HO + S        # W-conv moving width per chunk (112+S)

    nc = bacc.Bacc("TRN2", target_bir_lowering=False, debug=False)
    x_p = nc.declare_dram_parameter("x", [2, HT, C, W], bf16, isOutput=False)
    mh_p = nc.declare_dram_parameter("mh", [MR, C, HO], bf16, isOutput=False)
    mw_p = nc.declare_dram_parameter("mw", [HO, C, MR], fp8, isOutput=False)
    id_p = nc.declare_dram_parameter("ident", [HT, HT], bf16, isOutput=False)
    out_p = nc.declare_dram_parameter("out", [2, HO, C, W], bf16, isOutput=True)

    G = 16  # channels per DMA / store group
    with TileContext(nc) as tc:
        with tc.tile_pool(name="const", bufs=1) as constp, \
             tc.tile_pool(name="xg", bufs=3) as xgp, \
             tc.tile_pool(name="xt", bufs=3) as xtp, \
             tc.tile_pool(name="og", bufs=3) as ogp, \
             tc.tile_pool(name="pp", bufs=2, space="PSUM") as ppp, \
             tc.tile_pool(name="po", bufs=3, space="PSUM") as pop:
            ident = constp.tile([HT, HT], bf16)
            nc.sync.dma_start(out=ident[:, :], in_=id_p[:, :])
            mh = constp.tile([MR, C, HO], bf16, tag="mh")
            mw = constp.tile([HO, C, MR], fp8, tag="mw")
            ncopy = 0
            # small leading groups so the first matmuls start ~2us in
            # instead of waiting for a full 16-channel load
            sizes = [4, 4, 8] + [G] * ((C - 16) // G)
            c0 = 0
            for g, gs in enumerate(sizes):
                # chunked master loads so the first channels start early
                nc.sync.dma_start(out=mh[:, c0:c0 + gs, :], in_=mh_p[:, c0:c0 + gs, :])
                nc.sync.dma_start(out=mw[:, c0:c0 + gs, :], in_=mw_p[:, c0:c0 + gs, :])
                xg = []
                for t in (0, 1):
                    xg_t = xgp.tile([HT, G, W], bf16, tag=f"xg{t}")
                    nc.sync.dma_start(
                        out=xg_t[:, 0:gs, :], in_=x_p[t, :, c0:c0 + gs, :]
                    )
                    xg.append(xg_t)
                og0 = ogp.tile([HO, G, W], bf16, tag="og0")
                og1 = ogp.tile([HO, G, W], bf16, tag="og1")
                og = [og0, og1]
                po = [None, None]

                def emit_transposes(cl):
                    # transpose both w-chunks of both blocks: pp[:, 2t+q, :]
                    pp = ppp.tile([HO, 4, HT], bf16, name=f"pp_{g}_{cl}", tag="pp")
                    for t in (0, 1):
                        for q in (0, 1):
                            nc.tensor.matmul(
                                out=pp[:, 2 * t + q, :],
                                lhsT=xg[t][0:HT, cl, q * HO:(q + 1) * HO],
                                rhs=ident[:, :],
                                is_transpose=True,
                                skip_group_check=True,
                            )
                    xt = xtp.tile([HO, 4, HO], bf16, name=f"xt_{g}_{cl}", tag="xt")
                    nc.vector.tensor_copy(out=xt[:, :, :], in_=pp[:, :, S:S + HO])
                    return xt

                # software pipeline: transposes run one channel ahead so the
                # PE never waits on the DVE PSUM->SBUF copy of x^T
                xt_next = emit_transposes(0)
                for cl in range(gs):
                    c = c0 + cl
                    xt = xt_next
                    if cl + 1 < gs:
                        xt_next = emit_transposes(cl + 1)
                    if cl % 2 == 0:
                        po_t0 = pop.tile([HO, 2, 256], f32, tag="po0")
                        po_t1 = pop.tile([HO, 2, 256], f32, tag="po1")
                        po = [po_t0, po_t1]
                    sl = cl % 2
                    for t in (0, 1):
                        # H-conv (+identity): banded [HT->HO] stationary,
                        # x block moving (zero pad rows contribute nothing)
                        nc.tensor.matmul(
                            out=po[t][:, sl, 0:W],
                            lhsT=mh[0:HT, c, :],
                            rhs=xg[t][0:HT, cl, :],
                            start=True, stop=False,
                        )
                    for t in (0, 1):
                        # W-conv: transposed-x stationary, W master moving;
                        # two w_in chunks
                        nc.tensor.matmul(
                            out=po[t][:, sl, 0:WS],
                            lhsT=xt[0:HO, 2 * t, :],
                            rhs=mw[0:HO, c, S:S + WS],
                            start=False, stop=False,
                        )
                        nc.tensor.matmul(
                            out=po[t][:, sl, HO - S:W],
                            lhsT=xt[0:HO, 2 * t + 1, :],
                            rhs=mw[0:HO, c, 0:WS],
                            start=False, stop=True,
                        )
                    if cl % 2 == 1:
                        for t in (0, 1):
                            src = po[t][:, :, 0:W]
                            dst = og[t][:, cl - 1:cl + 1, :]
                            # GPSIMD cannot read PSUM; balance DVE vs ACT
                            if ncopy % 7 == 6:
                                nc.vector.tensor_copy(out=dst, in_=src)
                            else:
                                nc.scalar.copy(out=dst, in_=src)
                            ncopy += 1
                for t in (0, 1):
                    # stores go through SWDGE on the otherwise-idle gpsimd
                    # queue so they block neither loads (SP) nor copies (ACT);
                    # sub-group chunks shorten the end-of-kernel store tail
                    last = g == len(sizes) - 1
                    cm = 4 if (gs <= 8 or last) else gs // 2
                    for hf in range(gs // cm):
                        eng = nc.scalar if last else nc.gpsimd
                        eng.dma_start(
                            out=out_p[t, :, c0 + hf * cm:c0 + (hf + 1) * cm, :],
                            in_=og[t][:, hf * cm:(hf + 1) * cm, :],
                        )
                c0 += gs
    nc.compile()
    return nc


def _prepare_consts(weight_h, weight_w, r):
    r_val = float(max(np.float32(r), np.float32(1.0)))
    S = int(np.floor(3.0 * r_val)) + 1
    assert S <= 16, f"dilation r={r_val} too large for this kernel (S={S})"
    wh = np.asarray(weight_h)[:, 0, :, 0].astype(np.float64)
    ww = np.asarray(weight_w)[:, 0, 0, :].astype(np.float64)
    ah = _tap_coeffs(wh, r_val, S)
    aw = _tap_coeffs(ww, r_val, S)
    # identity rides the H master only (bf16 keeps it at ~2^-9 precision;
    # the W master is fp8 where a folded identity would cost ~3%)
    ah[:, S] += 1.0
    mh = _banded(ah, HO + 2 * S, HO, -S, S).astype(BF16)
    mw = _banded(aw, HO, HO + 2 * S, S, S).astype(ml_dtypes.float8_e4m3fn)
    ident = np.eye(HO + 2 * S, dtype=BF16)
    return S, mh, mw, ident


def kernel(x, weight_h, weight_w, r):
    from concourse.bass_utils import run_bass_kernel_spmd

    x = np.asarray(x, dtype=np.float32)
    assert x.shape == (B, C, H, W), x.shape
    S, mh, mw, ident = _prepare_consts(weight_h, weight_w, r)
    HT = HO + 2 * S

    if S not in _CACHE:
        _CACHE[S] = _build_nc(S)
    nc = _CACHE[S]

    xb = x.astype(BF16)
    in_maps = []
    for b in range(B):
        pk = np.zeros((2, HT, C, W), dtype=BF16)
        pk[0, S:HT] = xb[b, :, 0:HO + S].transpose(1, 0, 2)
        pk[1, 0:HO + S] = xb[b, :, HO - S:H].transpose(1, 0, 2)
        in_maps.append({"x": pk, "mh": mh, "mw": mw, "ident": ident})

    res = run_bass_kernel_spmd(nc, in_maps, core_ids=list(range(B)))
    out = np.empty((B, C, H, W), dtype=np.float32)
    for b in range(B):
        o = np.asarray(res.results[b]["out"])  # (2, HO, C, W) bf16
        out[b, :, 0:HO] = o[0].transpose(1, 0, 2)
        out[b, :, HO:H] = o[1].transpose(1, 0, 2)
    return out


# revision 14
# speedup vs baseline: 1.0980x; 1.0136x over previous
"""Trainium2 Bass kernel for DeformAxialDW.

Reference computes out = x + convH(x) + convW(x): depthwise 7-tap 1D convs
along H and W with fractional dilation r (bilinear sampling), which expand
into per-channel banded (Toeplitz) convs with 2S+1 integer taps,
S = floor(3*r)+1.

Layout/precision plan (per core = one batch item, 8 cores):
  - x is packed on the HOST to bf16 [2, 112+S, C, W]: two h-blocks with 2S
    rows of overlap (rows [0,112+S) and [112-S,224)).  The overlap lets each
    output block's H-conv be a single [112+S -> 112] banded matmul with NO
    edge/corner matmuls.
  - The identity (+x) is folded into the Toeplitz masters (+0.5 on the
    center tap of both the H and W masters), so out = Hconv' + Wconv'
    accumulates entirely in PSUM; no separate add pass.
  - One H master MH [112+2S, C, 112] is shared by both blocks via partition
    slices; one W master MW [112, C, 112+2S] is shared by both w-chunks via
    free-dim slices.
  - W-conv needs x transposed: 4 PE transposes per channel (bf16, via
    permutation matmul) -> PSUM -> one DVE copy to SBUF; the transposed
    chunks are the matmul *stationary* (stationary load is cheap), with the
    W master as the moving operand.
  - PSUM po tiles hold 2 channels padded to 256 f32 each (1 bank, no
    matmul bank crossing); f32->bf16 output copies run mostly on GpSimd
    (best cost/elem), every 4th on DVE.
  - Output bf16 [2, 112, C, W], unpacked + upcast on the host.
"""

import sys

import numpy as np

sys.path.insert(0, "/opt/trn_rl_repo")

import ml_dtypes

BF16 = ml_dtypes.bfloat16

C, H, W = 128, 224, 224
B = 8
HO = 112  # output rows per h-block

_CACHE = {}


def _tap_coeffs(w_taps: np.ndarray, r_val: float, S: int) -> np.ndarray:
    """Expand 7 fractional-dilation taps into 2S+1 integer-shift coeffs."""
    Cn, K = w_taps.shape
    P = K // 2
    alpha = np.zeros((Cn, 2 * S + 1), dtype=np.float64)
    for i in range(K):
        k_pos = i - P
        delta = np.float32(k_pos) * np.float32(r_val)
        d0 = int(np.floor(delta))
        frac = float(np.float32(delta) - np.float32(d0))
        alpha[:, d0 + S] += (1.0 - frac) * w_taps[:, i].astype(np.float64)
        alpha[:, d0 + 1 + S] += frac * w_taps[:, i].astype(np.float64)
    return alpha


def _banded(alpha: np.ndarray, rows: int, cols: int, diag_off: int, S: int):
    """M[i, c, jj] = alpha[c, (i - jj + diag_off) + S] where |i-jj+diag_off|<=S."""
    Cn = alpha.shape[0]
    out = np.zeros((rows, Cn, cols), dtype=np.float64)
    i = np.arange(rows)[:, None]
    jj = np.arange(cols)[None, :]
    d = i - jj + diag_off
    mask = np.abs(d) <= S
    ii, jjj = np.nonzero(mask)
    out[ii, :, jjj] = alpha[:, d[ii, jjj] + S].T
    return out


def _build_nc(S: int):
    import concourse.mybir as mybir
    from concourse import bacc
    from concourse.tile import TileContext

    f32 = mybir.dt.float32
    bf16 = mybir.dt.bfloat16
    fp8 = mybir.dt.float8e4

    HT = HO + 2 * S    # x tile rows per block incl. S zero-pad rows (124)
    MR = HO + 2 * S    # H master rows / W master cols (112+2S)
    WS = HO + S        # W-conv moving width per chunk (112+S)

    nc = bacc.Bacc("TRN2", target_bir_lowering=False, debug=False)
    x_p = nc.declare_dram_parameter("x", [2, HT, C, W], bf16, isOutput=False)
    mh_p = nc.declare_dram_parameter("mh", [MR, C, HO], bf16, isOutput=False)
    mw_p = nc.declare_dram_parameter("mw", [HO, C, MR], fp8, isOutput=False)
    id_p = nc.declare_dram_parameter("ident", [HT, HT], bf16, isOutput=False)
    out_p = nc.declare_dram_parameter("out", [2, HO, C, W], bf16, isOutput=True)

    G = 16  # channels per DMA / store group
    with TileContext(nc) as tc:
        with tc.tile_pool(name="const", bufs=1) as constp, \
             tc.tile_pool(name="xg", bufs=3) as xgp, \
             tc.tile_pool(name="xt", bufs=3) as xtp, \
             tc.tile_pool(name="og", bufs=3) as ogp, \
             tc.tile_pool(name="pp", bufs=2, space="PSUM") as ppp, \
             tc.tile_pool(name="po", bufs=3, space="PSUM") as pop:
            ident = constp.tile([HT, HT], bf16)
            nc.sync.dma_start(out=ident[:, :], in_=id_p[:, :])
            mh = constp.tile([MR, C, HO], bf16, tag="mh")
            mw = constp.tile([HO, C, MR], fp8, tag="mw")
            ncopy = 0
            # small leading groups so the first matmuls start ~2us in
            # instead of waiting for a full 16-channel load
            sizes = [4, 4, 8] + [G] * ((C - 16) // G)
            c0 = 0
            for g, gs in enumerate(sizes):
                # chunked master loads so the first channels start early
                nc.sync.dma_start(out=mh[:, c0:c0 + gs, :], in_=mh_p[:, c0:c0 + gs, :])
                nc.sync.dma_start(out=mw[:, c0:c0 + gs, :], in_=mw_p[:, c0:c0 + gs, :])
                xg = []
                for t in (0, 1):
                    xg_t = xgp.tile([HT, G, W], bf16, tag=f"xg{t}")
                    nc.sync.dma_start(
                        out=xg_t[:, 0:gs, :], in_=x_p[t, :, c0:c0 + gs, :]
                    )
                    xg.append(xg_t)
                og0 = ogp.tile([HO, G, W], bf16, tag="og0")
                og1 = ogp.tile([HO, G, W], bf16, tag="og1")
                og = [og0, og1]
                po = [None, None]

                def emit_transposes(cl):
                    # transpose both w-chunks of both blocks: pp[:, 2t+q, :]
                    pp = ppp.tile([HO, 4, HT], bf16, name=f"pp_{g}_{cl}", tag="pp")
                    for t in (0, 1):
                        for q in (0, 1):
                            nc.tensor.matmul(
                                out=pp[:, 2 * t + q, :],
                                lhsT=xg[t][0:HT, cl, q * HO:(q + 1) * HO],
                                rhs=ident[:, :],
                                is_transpose=True,
                                skip_group_check=True,
                            )
                    xt = xtp.tile([HO, 4, HO], bf16, name=f"xt_{g}_{cl}", tag="xt")
                    nc.vector.tensor_copy(out=xt[:, :, :], in_=pp[:, :, S:S + HO])
                    return xt

                # software pipeline: transposes run one channel ahead so the
                # PE never waits on the DVE PSUM->SBUF copy of x^T
                xt_next = emit_transposes(0)
                for cl in range(gs):
                    c = c0 + cl
                    xt = xt_next
                    if cl + 1 < gs:
                        xt_next = emit_transposes(cl + 1)
                    if cl % 2 == 0:
                        po_t0 = pop.tile([HO, 2, 256], f32, tag="po0")
                        po_t1 = pop.tile([HO, 2, 256], f32, tag="po1")
                        po = [po_t0, po_t1]
                    sl = cl % 2
                    for t in (0, 1):
                        # H-conv (+identity): banded [HT->HO] stationary,
                        # x block moving (zero pad rows contribute nothing)
                        nc.tensor.matmul(
                            out=po[t][:, sl, 0:W],
                            lhsT=mh[0:HT, c, :],
                            rhs=xg[t][0:HT, cl, :],
                            start=True, stop=False,
                        )
                    for t in (0, 1):
                        # W-conv: transposed-x stationary, W master moving;
                        # two w_in chunks
                        nc.tensor.matmul(
                            out=po[t][:, sl, 0:WS],
                            lhsT=xt[0:HO, 2 * t, :],
                            rhs=mw[0:HO, c, S:S + WS],
                            start=False, stop=False,
                        )
                        nc.tensor.matmul(
                            out=po[t][:, sl, HO - S:W],
                            lhsT=xt[0:HO, 2 * t + 1, :],
                            rhs=mw[0:HO, c, 0:WS],
                            start=False, stop=True,
                        )
                    if cl % 2 == 1:
                        for t in (0, 1):
                            src = po[t][:, :, 0:W]
                            dst = og[t][:, cl - 1:cl + 1, :]
                            # GPSIMD cannot read PSUM; balance DVE vs ACT
                            if ncopy % 7 == 6:
                                nc.vector.tensor_copy(out=dst, in_=src)
                            else:
                                nc.scalar.copy(out=dst, in_=src)
                            ncopy += 1
                for t in (0, 1):
                    # stores go through SWDGE on the otherwise-idle gpsimd
                    # queue so they block neither loads (SP) nor copies (ACT);
                    # sub-group chunks shorten the end-of-kernel store tail
                    cm = 4 if (gs <= 8 or g == len(sizes) - 1) else gs // 2
                    for hf in range(gs // cm):
                        nc.gpsimd.dma_start(
                            out=out_p[t, :, c0 + hf * cm:c0 + (hf + 1) * cm, :],
                            in_=og[t][:, hf * cm:(hf + 1) * cm, :],
                        )
                c0 += gs
    nc.compile()
    return nc


def _prepare_consts(weight_h, weight_w, r):
    r_val = float(max(np.float32(r), np.float32(1.0)))
    S = int(np.floor(3.0 * r_val)) + 1
    assert S <= 16, f"dilation r={r_val} too large for this kernel (S={S})"
    wh = np.asarray(weight_h)[:, 0, :, 0].astype(np.float64)
    ww = np.asarray(weight_w)[:, 0, 0, :].astype(np.float64)
    ah = _tap_coeffs(wh, r_val, S)
    aw = _tap_coeffs(ww, r_val, S)
    # identity rides the H master only (bf16 keeps it at ~2^-9 precision;
    # the W master is fp8 where a folded identity would cost ~3%)
    ah[:, S] += 1.0
    mh = _banded(ah, HO + 2 * S, HO, -S, S).astype(BF16)
    mw = _banded(aw, HO, HO + 2 * S, S, S).astype(ml_dtypes.float8_e4m3fn)
    ident = np.eye(HO + 2 * S, dtype=BF16)
    return S, mh, mw, ident


def kernel(x, weight_h, weight_w, r):
    from concourse.bass_utils import run_bass_kernel_spmd

    x = np.asarray(x, dtype=np.float32)
    assert x.shape == (B, C, H, W), x.shape
    S, mh, mw, ident = _prepare_consts(weight_h, weight_w, r)
    HT = HO + 2 * S

    if S not in _CACHE:
        _CACHE[S] = _build_nc(S)
    nc = _CACHE[S]

    xb = x.astype(BF16)
    in_maps = []
    for b in range(B):
        pk = np.zeros((2, HT, C, W), dtype=BF16)
        pk[0, S:HT] = xb[b, :, 0:HO + S].transpose(1, 0, 2)
        pk[1, 0:HO + S] = xb[b, :, HO - S:H].transpose(1, 0, 2)
        in_maps.append({"x": pk, "mh": mh, "mw": mw, "ident": ident})

    res = run_bass_kernel_spmd(nc, in_maps, core_ids=list(range(B)))
    out = np.empty((B, C, H, W), dtype=np.float32)
    for b in range(B):
        o = np.asarray(res.results[b]["out"])  # (2, HO, C, W) bf16
        out[b, :, 0:HO] = o[0].transpose(1, 0, 2)
        out[b, :, HO:H] = o[1].transpose(1, 0, 2)
    return out


# revision 16
# speedup vs baseline: 1.1059x; 1.0073x over previous
"""Trainium2 Bass kernel for DeformAxialDW.

Reference computes out = x + convH(x) + convW(x): depthwise 7-tap 1D convs
along H and W with fractional dilation r (bilinear sampling), which expand
into per-channel banded (Toeplitz) convs with 2S+1 integer taps,
S = floor(3*r)+1.

Layout/precision plan (per core = one batch item, 8 cores):
  - x is packed on the HOST to bf16 [2, 112+S, C, W]: two h-blocks with 2S
    rows of overlap (rows [0,112+S) and [112-S,224)).  The overlap lets each
    output block's H-conv be a single [112+S -> 112] banded matmul with NO
    edge/corner matmuls.
  - The identity (+x) is folded into the Toeplitz masters (+0.5 on the
    center tap of both the H and W masters), so out = Hconv' + Wconv'
    accumulates entirely in PSUM; no separate add pass.
  - One H master MH [112+2S, C, 112] is shared by both blocks via partition
    slices; one W master MW [112, C, 112+2S] is shared by both w-chunks via
    free-dim slices.
  - W-conv needs x transposed: 4 PE transposes per channel (bf16, via
    permutation matmul) -> PSUM -> one DVE copy to SBUF; the transposed
    chunks are the matmul *stationary* (stationary load is cheap), with the
    W master as the moving operand.
  - PSUM po tiles hold 2 channels padded to 256 f32 each (1 bank, no
    matmul bank crossing); f32->bf16 output copies run mostly on GpSimd
    (best cost/elem), every 4th on DVE.
  - Output bf16 [2, 112, C, W], unpacked + upcast on the host.
"""

import sys

import numpy as np

sys.path.insert(0, "/opt/trn_rl_repo")

import ml_dtypes

BF16 = ml_dtypes.bfloat16

C, H, W = 128, 224, 224
B = 8
HO = 112  # output rows per h-block

_CACHE = {}


def _tap_coeffs(w_taps: np.ndarray, r_val: float, S: int) -> np.ndarray:
    """Expand 7 fractional-dilation taps into 2S+1 integer-shift coeffs."""
    Cn, K = w_taps.shape
    P = K // 2
    alpha = np.zeros((Cn, 2 * S + 1), dtype=np.float64)
    for i in range(K):
        k_pos = i - P
        delta = np.float32(k_pos) * np.float32(r_val)
        d0 = int(np.floor(delta))
        frac = float(np.float32(delta) - np.float32(d0))
        alpha[:, d0 + S] += (1.0 - frac) * w_taps[:, i].astype(np.float64)
        alpha[:, d0 + 1 + S] += frac * w_taps[:, i].astype(np.float64)
    return alpha


def _banded(alpha: np.ndarray, rows: int, cols: int, diag_off: int, S: int):
    """M[i, c, jj] = alpha[c, (i - jj + diag_off) + S] where |i-jj+diag_off|<=S."""
    Cn = alpha.shape[0]
    out = np.zeros((rows, Cn, cols), dtype=np.float64)
    i = np.arange(rows)[:, None]
    jj = np.arange(cols)[None, :]
    d = i - jj + diag_off
    mask = np.abs(d) <= S
    ii, jjj = np.nonzero(mask)
    out[ii, :, jjj] = alpha[:, d[ii, jjj] + S].T
    return out


def _build_nc(S: int):
    import concourse.mybir as mybir
    from concourse import bacc
    from concourse.tile import TileContext

    f32 = mybir.dt.float32
    bf16 = mybir.dt.bfloat16
    fp8 = mybir.dt.float8e4

    HT = HO + 2 * S    # x tile rows per block incl. S zero-pad rows (124)
    MR = HO + 2 * S    # H master rows / W master cols (112+2S)
    WS = HO + S        # W-conv moving width per chunk (112+S)

    nc = bacc.Bacc("TRN2", target_bir_lowering=False, debug=False)
    x_p = nc.declare_dram_parameter("x", [2, HT, C, W], bf16, isOutput=False)
    mh_p = nc.declare_dram_parameter("mh", [MR, C, HO], bf16, isOutput=False)
    mw_p = nc.declare_dram_parameter("mw", [HO, C, MR], fp8, isOutput=False)
    id_p = nc.declare_dram_parameter("ident", [HT, HT], bf16, isOutput=False)
    out_p = nc.declare_dram_parameter("out", [2, HO, C, W], bf16, isOutput=True)

    G = 16  # channels per DMA / store group
    with TileContext(nc) as tc:
        with tc.tile_pool(name="const", bufs=1) as constp, \
             tc.tile_pool(name="xg", bufs=3) as xgp, \
             tc.tile_pool(name="xt", bufs=3) as xtp, \
             tc.tile_pool(name="og", bufs=3) as ogp, \
             tc.tile_pool(name="pp", bufs=2, space="PSUM") as ppp, \
             tc.tile_pool(name="po", bufs=3, space="PSUM") as pop:
            ident = constp.tile([HT, HT], bf16)
            nc.sync.dma_start(out=ident[:, :], in_=id_p[:, :])
            mh = constp.tile([MR, C, HO], bf16, tag="mh")
            mw = constp.tile([HO, C, MR], fp8, tag="mw")
            ncopy = 0
            # small leading groups so the first matmuls start early, and a
            # small trailing group so the final stores have a short tail
            sizes = [G] * (C // G)
            c0 = 0
            for g, gs in enumerate(sizes):
                xg = []
                for t in (0, 1):
                    xg_t = xgp.tile([HT, G, W], bf16, tag=f"xg{t}")
                    nc.sync.dma_start(
                        out=xg_t[:, 0:gs, :], in_=x_p[t, :, c0:c0 + gs, :]
                    )
                    xg.append(xg_t)
                # chunked master loads so the first channels start early
                nc.sync.dma_start(out=mh[:, c0:c0 + gs, :], in_=mh_p[:, c0:c0 + gs, :])
                nc.sync.dma_start(out=mw[:, c0:c0 + gs, :], in_=mw_p[:, c0:c0 + gs, :])
                og0 = ogp.tile([HO, G, W], bf16, tag="og0")
                og1 = ogp.tile([HO, G, W], bf16, tag="og1")
                og = [og0, og1]
                po = [None, None]

                def emit_transposes(cl):
                    # transpose both w-chunks of both blocks: pp[:, 2t+q, :]
                    pp = ppp.tile([HO, 4, HT], bf16, name=f"pp_{g}_{cl}", tag="pp")
                    for t in (0, 1):
                        for q in (0, 1):
                            nc.tensor.matmul(
                                out=pp[:, 2 * t + q, :],
                                lhsT=xg[t][0:HT, cl, q * HO:(q + 1) * HO],
                                rhs=ident[:, :],
                                is_transpose=True,
                                skip_group_check=True,
                            )
                    xt = xtp.tile([HO, 4, HO], bf16, name=f"xt_{g}_{cl}", tag="xt")
                    nc.vector.tensor_copy(out=xt[:, :, :], in_=pp[:, :, S:S + HO])
                    return xt

                # software pipeline: transposes run one channel ahead so the
                # PE never waits on the DVE PSUM->SBUF copy of x^T
                xt_next = emit_transposes(0)
                for cl in range(gs):
                    c = c0 + cl
                    xt = xt_next
                    if cl + 1 < gs:
                        xt_next = emit_transposes(cl + 1)
                    if cl % 2 == 0:
                        po_t0 = pop.tile([HO, 2, 256], f32, tag="po0")
                        po_t1 = pop.tile([HO, 2, 256], f32, tag="po1")
                        po = [po_t0, po_t1]
                    sl = cl % 2
                    for t in (0, 1):
                        # H-conv (+identity): banded [HT->HO] stationary,
                        # x block moving (zero pad rows contribute nothing)
                        nc.tensor.matmul(
                            out=po[t][:, sl, 0:W],
                            lhsT=mh[0:HT, c, :],
                            rhs=xg[t][0:HT, cl, :],
                            start=True, stop=False,
                        )
                    for t in (0, 1):
                        # W-conv: transposed-x stationary, W master moving;
                        # two w_in chunks
                        nc.tensor.matmul(
                            out=po[t][:, sl, 0:WS],
                            lhsT=xt[0:HO, 2 * t, :],
                            rhs=mw[0:HO, c, S:S + WS],
                            start=False, stop=False,
                        )
                        nc.tensor.matmul(
                            out=po[t][:, sl, HO - S:W],
                            lhsT=xt[0:HO, 2 * t + 1, :],
                            rhs=mw[0:HO, c, 0:WS],
                            start=False, stop=True,
                        )
                    if cl % 2 == 1:
                        for t in (0, 1):
                            src = po[t][:, :, 0:W]
                            dst = og[t][:, cl - 1:cl + 1, :]
                            # GPSIMD cannot read PSUM; balance DVE vs ACT
                            if ncopy % 7 == 6:
                                nc.vector.tensor_copy(out=dst, in_=src)
                            else:
                                nc.scalar.copy(out=dst, in_=src)
                            ncopy += 1
                for t in (0, 1):
                    # stores go through SWDGE on the otherwise-idle gpsimd
                    # queue so they block neither loads (SP) nor copies (ACT);
                    # sub-group chunks shorten the end-of-kernel store tail
                    cm = gs // 2 if gs > 8 else gs
                    for hf in range(gs // cm):
                        nc.gpsimd.dma_start(
                            out=out_p[t, :, c0 + hf * cm:c0 + (hf + 1) * cm, :],
                            in_=og[t][:, hf * cm:(hf + 1) * cm, :],
                        )
                c0 += gs
    nc.compile()
    return nc


def _prepare_consts(weight_h, weight_w, r):
    r_val = float(max(np.float32(r), np.float32(1.0)))
    S = int(np.floor(3.0 * r_val)) + 1
    assert S <= 16, f"dilation r={r_val} too large for this kernel (S={S})"
    wh = np.asarray(weight_h)[:, 0, :, 0].astype(np.float64)
    ww = np.asarray(weight_w)[:, 0, 0, :].astype(np.float64)
    ah = _tap_coeffs(wh, r_val, S)
    aw = _tap_coeffs(ww, r_val, S)
    # identity rides the H master only (bf16 keeps it at ~2^-9 precision;
    # the W master is fp8 where a folded identity would cost ~3%)
    ah[:, S] += 1.0
    mh = _banded(ah, HO + 2 * S, HO, -S, S).astype(BF16)
    mw = _banded(aw, HO, HO + 2 * S, S, S).astype(ml_dtypes.float8_e4m3fn)
    ident = np.eye(HO + 2 * S, dtype=BF16)
    return S, mh, mw, ident


def kernel(x, weight_h, weight_w, r):
    from concourse.bass_utils import run_bass_kernel_spmd

    x = np.asarray(x, dtype=np.float32)
    assert x.shape == (B, C, H, W), x.shape
    S, mh, mw, ident = _prepare_consts(weight_h, weight_w, r)
    HT = HO + 2 * S

    if S not in _CACHE:
        _CACHE[S] = _build_nc(S)
    nc = _CACHE[S]

    xb = x.astype(BF16)
    in_maps = []
    for b in range(B):
        pk = np.zeros((2, HT, C, W), dtype=BF16)
        pk[0, S:HT] = xb[b, :, 0:HO + S].transpose(1, 0, 2)
        pk[1, 0:HO + S] = xb[b, :, HO - S:H].transpose(1, 0, 2)
        in_maps.append({"x": pk, "mh": mh, "mw": mw, "ident": ident})

    res = run_bass_kernel_spmd(nc, in_maps, core_ids=list(range(B)))
    out = np.empty((B, C, H, W), dtype=np.float32)
    for b in range(B):
        o = np.asarray(res.results[b]["out"])  # (2, HO, C, W) bf16
        out[b, :, 0:HO] = o[0].transpose(1, 0, 2)
        out[b, :, HO:H] = o[1].transpose(1, 0, 2)
    return out


# revision 17
# speedup vs baseline: 1.1139x; 1.0072x over previous
"""Trainium2 Bass kernel for DeformAxialDW.

Reference computes out = x + convH(x) + convW(x): depthwise 7-tap 1D convs
along H and W with fractional dilation r (bilinear sampling), which expand
into per-channel banded (Toeplitz) convs with 2S+1 integer taps,
S = floor(3*r)+1.

Layout/precision plan (per core = one batch item, 8 cores):
  - x is packed on the HOST to bf16 [2, 112+S, C, W]: two h-blocks with 2S
    rows of overlap (rows [0,112+S) and [112-S,224)).  The overlap lets each
    output block's H-conv be a single [112+S -> 112] banded matmul with NO
    edge/corner matmuls.
  - The identity (+x) is folded into the Toeplitz masters (+0.5 on the
    center tap of both the H and W masters), so out = Hconv' + Wconv'
    accumulates entirely in PSUM; no separate add pass.
  - One H master MH [112+2S, C, 112] is shared by both blocks via partition
    slices; one W master MW [112, C, 112+2S] is shared by both w-chunks via
    free-dim slices.
  - W-conv needs x transposed: 4 PE transposes per channel (bf16, via
    permutation matmul) -> PSUM -> one DVE copy to SBUF; the transposed
    chunks are the matmul *stationary* (stationary load is cheap), with the
    W master as the moving operand.
  - PSUM po tiles hold 2 channels padded to 256 f32 each (1 bank, no
    matmul bank crossing); f32->bf16 output copies run mostly on GpSimd
    (best cost/elem), every 4th on DVE.
  - Output bf16 [2, 112, C, W], unpacked + upcast on the host.
"""

import sys

import numpy as np

sys.path.insert(0, "/opt/trn_rl_repo")

import ml_dtypes

BF16 = ml_dtypes.bfloat16

C, H, W = 128, 224, 224
B = 8
HO = 112  # output rows per h-block

_CACHE = {}


def _tap_coeffs(w_taps: np.ndarray, r_val: float, S: int) -> np.ndarray:
    """Expand 7 fractional-dilation taps into 2S+1 integer-shift coeffs."""
    Cn, K = w_taps.shape
    P = K // 2
    alpha = np.zeros((Cn, 2 * S + 1), dtype=np.float64)
    for i in range(K):
        k_pos = i - P
        delta = np.float32(k_pos) * np.float32(r_val)
        d0 = int(np.floor(delta))
        frac = float(np.float32(delta) - np.float32(d0))
        alpha[:, d0 + S] += (1.0 - frac) * w_taps[:, i].astype(np.float64)
        alpha[:, d0 + 1 + S] += frac * w_taps[:, i].astype(np.float64)
    return alpha


def _banded(alpha: np.ndarray, rows: int, cols: int, diag_off: int, S: int):
    """M[i, c, jj] = alpha[c, (i - jj + diag_off) + S] where |i-jj+diag_off|<=S."""
    Cn = alpha.shape[0]
    out = np.zeros((rows, Cn, cols), dtype=np.float64)
    i = np.arange(rows)[:, None]
    jj = np.arange(cols)[None, :]
    d = i - jj + diag_off
    mask = np.abs(d) <= S
    ii, jjj = np.nonzero(mask)
    out[ii, :, jjj] = alpha[:, d[ii, jjj] + S].T
    return out


def _build_nc(S: int):
    import concourse.mybir as mybir
    from concourse import bacc
    from concourse.tile import TileContext

    f32 = mybir.dt.float32
    bf16 = mybir.dt.bfloat16
    fp8 = mybir.dt.float8e4

    HT = HO + 2 * S    # x tile rows per block incl. S zero-pad rows (124)
    MR = HO + 2 * S    # H master rows / W master cols (112+2S)
    WS = HO + S        # W-conv moving width per chunk (112+S)

    nc = bacc.Bacc("TRN2", target_bir_lowering=False, debug=False)
    x_p = nc.declare_dram_parameter("x", [2, HT, C, W], bf16, isOutput=False)
    mh_p = nc.declare_dram_parameter("mh", [MR, C, HO], bf16, isOutput=False)
    mw_p = nc.declare_dram_parameter("mw", [HO, C, MR], fp8, isOutput=False)
    id_p = nc.declare_dram_parameter("ident", [HT, HT], bf16, isOutput=False)
    out_p = nc.declare_dram_parameter("out", [2, HO, C, W], bf16, isOutput=True)

    G = 16  # channels per DMA / store group
    with TileContext(nc) as tc:
        with tc.tile_pool(name="const", bufs=1) as constp, \
             tc.tile_pool(name="xg", bufs=3) as xgp, \
             tc.tile_pool(name="xt", bufs=3) as xtp, \
             tc.tile_pool(name="og", bufs=3) as ogp, \
             tc.tile_pool(name="pp", bufs=2, space="PSUM") as ppp, \
             tc.tile_pool(name="po", bufs=3, space="PSUM") as pop:
            ident = constp.tile([HT, HT], bf16)
            nc.sync.dma_start(out=ident[:, :], in_=id_p[:, :])
            mh = constp.tile([MR, C, HO], bf16, tag="mh")
            mw = constp.tile([HO, C, MR], fp8, tag="mw")
            ncopy = 0
            # small leading groups so the first matmuls start early, and a
            # small trailing group so the final stores have a short tail
            sizes = [8, 8] + [G] * ((C - 16) // G)
            c0 = 0
            for g, gs in enumerate(sizes):
                xg = []
                for t in (0, 1):
                    xg_t = xgp.tile([HT, G, W], bf16, tag=f"xg{t}")
                    nc.sync.dma_start(
                        out=xg_t[:, 0:gs, :], in_=x_p[t, :, c0:c0 + gs, :]
                    )
                    if g == 0 and t == 0:
                        # first group: H master chunk right after the first x
                        # block so the first H matmuls start ASAP
                        nc.sync.dma_start(
                            out=mh[:, c0:c0 + gs, :], in_=mh_p[:, c0:c0 + gs, :]
                        )
                    xg.append(xg_t)
                if g > 0:
                    nc.sync.dma_start(
                        out=mh[:, c0:c0 + gs, :], in_=mh_p[:, c0:c0 + gs, :]
                    )
                nc.sync.dma_start(out=mw[:, c0:c0 + gs, :], in_=mw_p[:, c0:c0 + gs, :])
                og0 = ogp.tile([HO, G, W], bf16, tag="og0")
                og1 = ogp.tile([HO, G, W], bf16, tag="og1")
                og = [og0, og1]
                po = [None, None]

                def emit_transposes(cl):
                    # transpose both w-chunks of both blocks: pp[:, 2t+q, :]
                    pp = ppp.tile([HO, 4, HT], bf16, name=f"pp_{g}_{cl}", tag="pp")
                    for t in (0, 1):
                        for q in (0, 1):
                            nc.tensor.matmul(
                                out=pp[:, 2 * t + q, :],
                                lhsT=xg[t][0:HT, cl, q * HO:(q + 1) * HO],
                                rhs=ident[:, :],
                                is_transpose=True,
                                skip_group_check=True,
                            )
                    xt = xtp.tile([HO, 4, HO], bf16, name=f"xt_{g}_{cl}", tag="xt")
                    nc.vector.tensor_copy(out=xt[:, :, :], in_=pp[:, :, S:S + HO])
                    return xt

                # software pipeline: transposes run one channel ahead so the
                # PE never waits on the DVE PSUM->SBUF copy of x^T
                xt_next = emit_transposes(0)
                for cl in range(gs):
                    c = c0 + cl
                    xt = xt_next
                    if cl + 1 < gs:
                        xt_next = emit_transposes(cl + 1)
                    if cl % 2 == 0:
                        po_t0 = pop.tile([HO, 2, 256], f32, tag="po0")
                        po_t1 = pop.tile([HO, 2, 256], f32, tag="po1")
                        po = [po_t0, po_t1]
                    sl = cl % 2
                    for t in (0, 1):
                        # H-conv (+identity): banded [HT->HO] stationary,
                        # x block moving (zero pad rows contribute nothing)
                        nc.tensor.matmul(
                            out=po[t][:, sl, 0:W],
                            lhsT=mh[0:HT, c, :],
                            rhs=xg[t][0:HT, cl, :],
                            start=True, stop=False,
                        )
                    for t in (0, 1):
                        # W-conv: transposed-x stationary, W master moving;
                        # two w_in chunks
                        nc.tensor.matmul(
                            out=po[t][:, sl, 0:WS],
                            lhsT=xt[0:HO, 2 * t, :],
                            rhs=mw[0:HO, c, S:S + WS],
                            start=False, stop=False,
                        )
                        nc.tensor.matmul(
                            out=po[t][:, sl, HO - S:W],
                            lhsT=xt[0:HO, 2 * t + 1, :],
                            rhs=mw[0:HO, c, 0:WS],
                            start=False, stop=True,
                        )
                    if cl % 2 == 1:
                        for t in (0, 1):
                            src = po[t][:, :, 0:W]
                            dst = og[t][:, cl - 1:cl + 1, :]
                            # GPSIMD cannot read PSUM; balance DVE vs ACT
                            if ncopy % 7 == 6:
                                nc.vector.tensor_copy(out=dst, in_=src)
                            else:
                                nc.scalar.copy(out=dst, in_=src)
                            ncopy += 1
                for t in (0, 1):
                    # stores go through SWDGE on the otherwise-idle gpsimd
                    # queue so they block neither loads (SP) nor copies (ACT);
                    # sub-group chunks shorten the end-of-kernel store tail
                    cm = gs // 2 if gs > 8 else gs
                    for hf in range(gs // cm):
                        nc.gpsimd.dma_start(
                            out=out_p[t, :, c0 + hf * cm:c0 + (hf + 1) * cm, :],
                            in_=og[t][:, hf * cm:(hf + 1) * cm, :],
                        )
                c0 += gs
    nc.compile()
    return nc


def _prepare_consts(weight_h, weight_w, r):
    r_val = float(max(np.float32(r), np.float32(1.0)))
    S = int(np.floor(3.0 * r_val)) + 1
    assert S <= 16, f"dilation r={r_val} too large for this kernel (S={S})"
    wh = np.asarray(weight_h)[:, 0, :, 0].astype(np.float64)
    ww = np.asarray(weight_w)[:, 0, 0, :].astype(np.float64)
    ah = _tap_coeffs(wh, r_val, S)
    aw = _tap_coeffs(ww, r_val, S)
    # identity rides the H master only (bf16 keeps it at ~2^-9 precision;
    # the W master is fp8 where a folded identity would cost ~3%)
    ah[:, S] += 1.0
    mh = _banded(ah, HO + 2 * S, HO, -S, S).astype(BF16)
    mw = _banded(aw, HO, HO + 2 * S, S, S).astype(ml_dtypes.float8_e4m3fn)
    ident = np.eye(HO + 2 * S, dtype=BF16)
    return S, mh, mw, ident


def kernel(x, weight_h, weight_w, r):
    from concourse.bass_utils import run_bass_kernel_spmd

    x = np.asarray(x, dtype=np.float32)
    assert x.shape == (B, C, H, W), x.shape
    S, mh, mw, ident = _prepare_consts(weight_h, weight_w, r)
    HT = HO + 2 * S

    if S not in _CACHE:
        _CACHE[S] = _build_nc(S)
    nc = _CACHE[S]

    xb = x.astype(BF16)
    in_maps = []
    for b in range(B):
        pk = np.zeros((2, HT, C, W), dtype=BF16)
        pk[0, S:HT] = xb[b, :, 0:HO + S].transpose(1, 0, 2)
        pk[1, 0:HO + S] = xb[b, :, HO - S:H].transpose(1, 0, 2)
        in_maps.append({"x": pk, "mh": mh, "mw": mw, "ident": ident})

    res = run_bass_kernel_spmd(nc, in_maps, core_ids=list(range(B)))
    out = np.empty((B, C, H, W), dtype=np.float32)
    for b in range(B):
        o = np.asarray(res.results[b]["out"])  # (2, HO, C, W) bf16
        out[b, :, 0:HO] = o[0].transpose(1, 0, 2)
        out[b, :, HO:H] = o[1].transpose(1, 0, 2)
    return out


# revision 18
# speedup vs baseline: 1.1184x; 1.0040x over previous
"""Trainium2 Bass kernel for DeformAxialDW.

Reference computes out = x + convH(x) + convW(x): depthwise 7-tap 1D convs
along H and W with fractional dilation r (bilinear sampling), which expand
into per-channel banded (Toeplitz) convs with 2S+1 integer taps,
S = floor(3*r)+1.

Layout/precision plan (per core = one batch item, 8 cores):
  - x is packed on the HOST to bf16 [2, 112+S, C, W]: two h-blocks with 2S
    rows of overlap (rows [0,112+S) and [112-S,224)).  The overlap lets each
    output block's H-conv be a single [112+S -> 112] banded matmul with NO
    edge/corner matmuls.
  - The identity (+x) is folded into the Toeplitz masters (+0.5 on the
    center tap of both the H and W masters), so out = Hconv' + Wconv'
    accumulates entirely in PSUM; no separate add pass.
  - One H master MH [112+2S, C, 112] is shared by both blocks via partition
    slices; one W master MW [112, C, 112+2S] is shared by both w-chunks via
    free-dim slices.
  - W-conv needs x transposed: 4 PE transposes per channel (bf16, via
    permutation matmul) -> PSUM -> one DVE copy to SBUF; the transposed
    chunks are the matmul *stationary* (stationary load is cheap), with the
    W master as the moving operand.
  - PSUM po tiles hold 2 channels padded to 256 f32 each (1 bank, no
    matmul bank crossing); f32->bf16 output copies run mostly on GpSimd
    (best cost/elem), every 4th on DVE.
  - Output bf16 [2, 112, C, W], unpacked + upcast on the host.
"""

import sys

import numpy as np

sys.path.insert(0, "/opt/trn_rl_repo")

import ml_dtypes

BF16 = ml_dtypes.bfloat16

C, H, W = 128, 224, 224
B = 8
HO = 112  # output rows per h-block

_CACHE = {}


def _tap_coeffs(w_taps: np.ndarray, r_val: float, S: int) -> np.ndarray:
    """Expand 7 fractional-dilation taps into 2S+1 integer-shift coeffs."""
    Cn, K = w_taps.shape
    P = K // 2
    alpha = np.zeros((Cn, 2 * S + 1), dtype=np.float64)
    for i in range(K):
        k_pos = i - P
        delta = np.float32(k_pos) * np.float32(r_val)
        d0 = int(np.floor(delta))
        frac = float(np.float32(delta) - np.float32(d0))
        alpha[:, d0 + S] += (1.0 - frac) * w_taps[:, i].astype(np.float64)
        alpha[:, d0 + 1 + S] += frac * w_taps[:, i].astype(np.float64)
    return alpha


def _banded(alpha: np.ndarray, rows: int, cols: int, diag_off: int, S: int):
    """M[i, c, jj] = alpha[c, (i - jj + diag_off) + S] where |i-jj+diag_off|<=S."""
    Cn = alpha.shape[0]
    out = np.zeros((rows, Cn, cols), dtype=np.float64)
    i = np.arange(rows)[:, None]
    jj = np.arange(cols)[None, :]
    d = i - jj + diag_off
    mask = np.abs(d) <= S
    ii, jjj = np.nonzero(mask)
    out[ii, :, jjj] = alpha[:, d[ii, jjj] + S].T
    return out


def _build_nc(S: int):
    import concourse.mybir as mybir
    from concourse import bacc
    from concourse.tile import TileContext

    f32 = mybir.dt.float32
    bf16 = mybir.dt.bfloat16
    fp8 = mybir.dt.float8e4

    HT = HO + 2 * S    # x tile rows per block incl. S zero-pad rows (124)
    MR = HO + 2 * S    # H master rows / W master cols (112+2S)
    WS = HO + S        # W-conv moving width per chunk (112+S)

    nc = bacc.Bacc("TRN2", target_bir_lowering=False, debug=False)
    x_p = nc.declare_dram_parameter("x", [2, HT, C, W], bf16, isOutput=False)
    mh_p = nc.declare_dram_parameter("mh", [MR, C, HO], bf16, isOutput=False)
    mw_p = nc.declare_dram_parameter("mw", [HO, C, MR], fp8, isOutput=False)
    id_p = nc.declare_dram_parameter("ident", [HT, HT], bf16, isOutput=False)
    out_p = nc.declare_dram_parameter("out", [2, HO, C, W], bf16, isOutput=True)

    G = 16  # channels per DMA / store group
    with TileContext(nc) as tc:
        with tc.tile_pool(name="const", bufs=1) as constp, \
             tc.tile_pool(name="xg", bufs=4) as xgp, \
             tc.tile_pool(name="xt", bufs=4) as xtp, \
             tc.tile_pool(name="og", bufs=3) as ogp, \
             tc.tile_pool(name="pp", bufs=2, space="PSUM") as ppp, \
             tc.tile_pool(name="po", bufs=3, space="PSUM") as pop:
            ident = constp.tile([HT, HT], bf16)
            nc.sync.dma_start(out=ident[:, :], in_=id_p[:, :])
            mh = constp.tile([MR, C, HO], bf16, tag="mh")
            mw = constp.tile([HO, C, MR], fp8, tag="mw")
            ncopy = 0
            # small leading groups so the first matmuls start early, and a
            # small trailing group so the final stores have a short tail
            sizes = [8, 8] + [G] * ((C - 16) // G)
            c0 = 0
            for g, gs in enumerate(sizes):
                xg = []
                for t in (0, 1):
                    xg_t = xgp.tile([HT, G, W], bf16, tag=f"xg{t}")
                    nc.sync.dma_start(
                        out=xg_t[:, 0:gs, :], in_=x_p[t, :, c0:c0 + gs, :]
                    )
                    if g == 0 and t == 0:
                        # first group: H master chunk right after the first x
                        # block so the first H matmuls start ASAP
                        nc.sync.dma_start(
                            out=mh[:, c0:c0 + gs, :], in_=mh_p[:, c0:c0 + gs, :]
                        )
                    xg.append(xg_t)
                if g > 0:
                    nc.sync.dma_start(
                        out=mh[:, c0:c0 + gs, :], in_=mh_p[:, c0:c0 + gs, :]
                    )
                nc.sync.dma_start(out=mw[:, c0:c0 + gs, :], in_=mw_p[:, c0:c0 + gs, :])
                og0 = ogp.tile([HO, G, W], bf16, tag="og0")
                og1 = ogp.tile([HO, G, W], bf16, tag="og1")
                og = [og0, og1]
                po = [None, None]

                def emit_transposes(cl):
                    # transpose both w-chunks of both blocks: pp[:, 2t+q, :]
                    pp = ppp.tile([HO, 4, HT], bf16, name=f"pp_{g}_{cl}", tag="pp")
                    for t in (0, 1):
                        for q in (0, 1):
                            nc.tensor.matmul(
                                out=pp[:, 2 * t + q, :],
                                lhsT=xg[t][0:HT, cl, q * HO:(q + 1) * HO],
                                rhs=ident[:, :],
                                is_transpose=True,
                                skip_group_check=True,
                            )
                    xt = xtp.tile([HO, 4, HO], bf16, name=f"xt_{g}_{cl}", tag="xt")
                    nc.vector.tensor_copy(out=xt[:, :, :], in_=pp[:, :, S:S + HO])
                    return xt

                # software pipeline: transposes run one channel ahead so the
                # PE never waits on the DVE PSUM->SBUF copy of x^T
                xt_next = emit_transposes(0)
                for cl in range(gs):
                    c = c0 + cl
                    xt = xt_next
                    if cl + 1 < gs:
                        xt_next = emit_transposes(cl + 1)
                    if cl % 2 == 0:
                        po_t0 = pop.tile([HO, 2, 256], f32, tag="po0")
                        po_t1 = pop.tile([HO, 2, 256], f32, tag="po1")
                        po = [po_t0, po_t1]
                    sl = cl % 2
                    for t in (0, 1):
                        # H-conv (+identity): banded [HT->HO] stationary,
                        # x block moving (zero pad rows contribute nothing)
                        nc.tensor.matmul(
                            out=po[t][:, sl, 0:W],
                            lhsT=mh[0:HT, c, :],
                            rhs=xg[t][0:HT, cl, :],
                            start=True, stop=False,
                        )
                    for t in (0, 1):
                        # W-conv: transposed-x stationary, W master moving;
                        # two w_in chunks
                        nc.tensor.matmul(
                            out=po[t][:, sl, 0:WS],
                            lhsT=xt[0:HO, 2 * t, :],
                            rhs=mw[0:HO, c, S:S + WS],
                            start=False, stop=False,
                        )
                        nc.tensor.matmul(
                            out=po[t][:, sl, HO - S:W],
                            lhsT=xt[0:HO, 2 * t + 1, :],
                            rhs=mw[0:HO, c, 0:WS],
                            start=False, stop=True,
                        )
                    if cl % 2 == 1:
                        for t in (0, 1):
                            src = po[t][:, :, 0:W]
                            dst = og[t][:, cl - 1:cl + 1, :]
                            # GPSIMD cannot read PSUM; balance DVE vs ACT
                            if ncopy % 7 == 6:
                                nc.vector.tensor_copy(out=dst, in_=src)
                            else:
                                nc.scalar.copy(out=dst, in_=src)
                            ncopy += 1
                for t in (0, 1):
                    # stores go through SWDGE on the otherwise-idle gpsimd
                    # queue so they block neither loads (SP) nor copies (ACT);
                    # sub-group chunks shorten the end-of-kernel store tail
                    if g == len(sizes) - 1:
                        cms = [8, 4, 4]
                    elif gs > 8:
                        cms = [gs // 2, gs // 2]
                    else:
                        cms = [gs]
                    cb = 0
                    for cm in cms:
                        nc.gpsimd.dma_start(
                            out=out_p[t, :, c0 + cb:c0 + cb + cm, :],
                            in_=og[t][:, cb:cb + cm, :],
                        )
                        cb += cm
                c0 += gs
    nc.compile()
    return nc


def _prepare_consts(weight_h, weight_w, r):
    r_val = float(max(np.float32(r), np.float32(1.0)))
    S = int(np.floor(3.0 * r_val)) + 1
    assert S <= 16, f"dilation r={r_val} too large for this kernel (S={S})"
    wh = np.asarray(weight_h)[:, 0, :, 0].astype(np.float64)
    ww = np.asarray(weight_w)[:, 0, 0, :].astype(np.float64)
    ah = _tap_coeffs(wh, r_val, S)
    aw = _tap_coeffs(ww, r_val, S)
    # identity rides the H master only (bf16 keeps it at ~2^-9 precision;
    # the W master is fp8 where a folded identity would cost ~3%)
    ah[:, S] += 1.0
    mh = _banded(ah, HO + 2 * S, HO, -S, S).astype(BF16)
    mw = _banded(aw, HO, HO + 2 * S, S, S).astype(ml_dtypes.float8_e4m3fn)
    ident = np.eye(HO + 2 * S, dtype=BF16)
    return S, mh, mw, ident


def kernel(x, weight_h, weight_w, r):
    from concourse.bass_utils import run_bass_kernel_spmd

    x = np.asarray(x, dtype=np.float32)
    assert x.shape == (B, C, H, W), x.shape
    S, mh, mw, ident = _prepare_consts(weight_h, weight_w, r)
    HT = HO + 2 * S

    if S not in _CACHE:
        _CACHE[S] = _build_nc(S)
    nc = _CACHE[S]

    xb = x.astype(BF16)
    in_maps = []
    for b in range(B):
        pk = np.zeros((2, HT, C, W), dtype=BF16)
        pk[0, S:HT] = xb[b, :, 0:HO + S].transpose(1, 0, 2)
        pk[1, 0:HO + S] = xb[b, :, HO - S:H].transpose(1, 0, 2)
        in_maps.append({"x": pk, "mh": mh, "mw": mw, "ident": ident})

    res = run_bass_kernel_spmd(nc, in_maps, core_ids=list(range(B)))
    out = np.empty((B, C, H, W), dtype=np.float32)
    for b in range(B):
        o = np.asarray(res.results[b]["out"])  # (2, HO, C, W) bf16
        out[b, :, 0:HO] = o[0].transpose(1, 0, 2)
        out[b, :, HO:H] = o[1].transpose(1, 0, 2)
    return out
